# revision 19
# baseline (speedup 1.0000x reference)
"""Trainium2 Bass kernel for nn_LuongAttention.

Reference math (per batch b):
    S   = Dec @ Enc^T          # [T_dec, T_enc]
    Out = S @ Enc              # [T_dec, D]

By associativity:  Out = Dec @ (Enc^T @ Enc) = Dec @ G with G = Enc^T Enc
a [D, D] = [128, 128] Gram matrix.  This removes the [2048, 2048]
intermediate entirely (16x less FLOPs) and makes the kernel
memory-bound: ~3 MiB HBM I/O per core.

Sharding: data-parallel over batch B=8 -> one batch per NeuronCore.

Device-side layout trick: the host feeds Dec pre-transposed (DecT
[D, T]) and receives Out transposed (OutT [D, T]); the host transposes
the result back during the gather (pure layout permutation, no math).
With that:
  - G = sum_i EncTile_i^T @ EncTile_i  (accumulating PE matmuls, natural
    encoder layout - no transposes needed)
  - OutT = G @ DecT computed as matmul(lhsT=G, rhs=DecT chunk) with wide
    moving chunks (G is symmetric so lhsT=G gives G.T@X = G@X)
  - no PE transposes, no identity, minimal PSUM->SBUF copies

With SPLIT_G (2-byte dtypes), G is built as two half-Grams and the
final product accumulates OutT = Ga@DecT + Gb@DecT in PSUM, letting the
wide final matmuls start once the first half of the encoder has landed.
"""

import os
import sys
from contextlib import ExitStack

import numpy as np

for _p in (
    "/opt/trn_rl_repo",
    "/root/.axon_site",
    "/root/.axon_site/_ro/trn_rl_repo",
    "/root/.axon_site/_ro/pypackages",
):
    if os.path.isdir(_p) and _p not in sys.path:
        sys.path.append(_p)

import concourse.bacc as bacc
import concourse.mybir as mybir
import concourse.tile as tile
from concourse.bass_utils import run_bass_kernel_spmd

B, T, D, P = 8, 2048, 128, 128
NT = T // P  # 16 row tiles of 128

# tunables
MM_DTYPE = "fp16"  # "fp32" | "f32r" | "bf16" | "fp16"
ENC_CHUNKS = 4  # tiles per chunk = NT / ENC_CHUNKS
DEC_CHUNKS = 2
FINAL_N = 512  # moving-operand width of the final matmul
COPY_N = 256  # PSUM->SBUF copy granularity
STORE_N = 256  # store granularity
WARMUP_MMS = 0  # junk matmuls issued early to trigger the PE HAM clock ramp
SPLIT_G = False  # 2-step final accumulation: OutT = Ga@DecT + Gb@DecT


def _build_nc(mm_dtype=None):
    mm_dtype = mm_dtype or MM_DTYPE
    nc = bacc.Bacc("TRN2", target_bir_lowering=False, debug=False)
    f32 = mybir.dt.float32
    f32r = mybir.dt.float32r
    bf16 = mybir.dt.bfloat16
    fp16 = mybir.dt.float16

    # float32r is 4-byte fp32 storage with reduced-precision PE multiplies;
    # the BIR verifier requires every producer of an f32r matmul input to
    # carry the f32r dtype, so DRAM/SBUF tensors are declared f32r
    # end-to-end (numpy side stays np.float32).
    in_dt = {"bf16": bf16, "fp16": fp16, "f32r": f32r}.get(mm_dtype, f32)
    split_g = SPLIT_G and mm_dtype in ("fp16", "bf16")

    enc_h = nc.dram_tensor("enc", [T, D], in_dt, kind="ExternalInput")
    dect_h = nc.dram_tensor("dect", [D, T], in_dt, kind="ExternalInput")
    out_h = nc.dram_tensor("out", [D, T], f32, kind="ExternalOutput")

    # [p, n, d] view of encoder (p = row within tile, n = tile index)
    enc_v = enc_h.ap().rearrange("(n p) d -> p n d", p=P)
    dect_v = dect_h.ap()
    out_v = out_h.ap()

    with ExitStack() as ctx:
        tc = ctx.enter_context(tile.TileContext(nc))
        singles = ctx.enter_context(tc.tile_pool(name="singles", bufs=1))
        psum = ctx.enter_context(tc.tile_pool(name="psum", bufs=3, space="PSUM"))
        gpsum = ctx.enter_context(tc.tile_pool(name="gpsum", bufs=1, space="PSUM"))

        enc_sb = singles.tile([P, NT, D], in_dt)
        dect_sb = singles.tile([P, T], in_dt)
        out_sb = singles.tile([P, T], f32)

        if WARMUP_MMS:
            # Busy the PE while the first loads are in flight so the HAM
            # clock gate releases (1.2 -> 2.4 GHz) before the real matmuls.
            wsrc = singles.tile([P, 512], bf16)
            nc.gpsimd.memset(wsrc[:], 0.0)
            wps = gpsum.tile([P, 512], f32, tag="warm")
            for w in range(WARMUP_MMS):
                nc.tensor.matmul(
                    wps[:],
                    lhsT=wsrc[:, :P],
                    rhs=wsrc[:],
                    start=(w == 0),
                    stop=(w == WARMUP_MMS - 1),
                )
            wsink = singles.tile([P, 1], f32)
            nc.vector.tensor_copy(wsink[:], wps[:, :1])

        # Interleave chunked loads across both HWDGE rings (SP=sync,
        # ACT=scalar); encoder first on each ring since G consumes it first.
        cs = NT // ENC_CHUNKS
        for c in range(ENC_CHUNKS):
            eng = nc.sync if c % 2 == 0 else nc.scalar
            eng.dma_start(
                out=enc_sb[:, c * cs : (c + 1) * cs, :],
                in_=enc_v[:, c * cs : (c + 1) * cs, :],
            )
        cs = T // DEC_CHUNKS
        for c in range(DEC_CHUNKS):
            eng = nc.sync if c % 2 == 0 else nc.scalar
            eng.dma_start(
                out=dect_sb[:, c * cs : (c + 1) * cs],
                in_=dect_v[:, c * cs : (c + 1) * cs],
            )

        # ---- Gram matrix construction ----
        if split_g:
            H = NT // 2
            g_parts = []
            for h in range(2):
                ps = gpsum.tile([P, P], f32, tag=f"g{h}")
                for i in range(H):
                    nc.tensor.matmul(
                        ps[:],
                        lhsT=enc_sb[:, h * H + i, :],
                        rhs=enc_sb[:, h * H + i, :],
                        start=(i == 0),
                        stop=(i == H - 1),
                    )
                gh = singles.tile([P, P], in_dt, tag=f"gsb{h}")
                nc.vector.tensor_copy(gh[:], ps[:])
                g_parts.append(gh)
        elif mm_dtype == "f32r":
            # Pair tiles: rhs spans two adjacent tiles (N=256) so float32r
            # runs at 1 cycle/row instead of 4.  Useful halves:
            #   ps_a left  half accumulates even-tile Grams
            #   ps_b right half accumulates odd-tile Grams
            g_sb = singles.tile([P, P], in_dt)
            ps_a = gpsum.tile([P, 2 * P], f32, tag="ga")
            ps_b = gpsum.tile([P, 2 * P], f32, tag="gb")
            for pair in range(NT // 2):
                i, j = 2 * pair, 2 * pair + 1
                rhs = enc_sb[:, i : i + 2, :]
                nc.tensor.matmul(
                    ps_a[:],
                    lhsT=enc_sb[:, i, :],
                    rhs=rhs,
                    start=(pair == 0),
                    stop=(pair == NT // 2 - 1),
                )
                nc.tensor.matmul(
                    ps_b[:],
                    lhsT=enc_sb[:, j, :],
                    rhs=rhs,
                    start=(pair == 0),
                    stop=(pair == NT // 2 - 1),
                )
            # DVE may read only one PSUM operand per instruction.
            ga_sb = singles.tile([P, P], f32)
            nc.vector.tensor_copy(ga_sb[:], ps_a[:, :P])
            nc.vector.tensor_add(g_sb[:], ga_sb[:], ps_b[:, P:])
            g_parts = [g_sb]
        else:
            g_sb = singles.tile([P, P], in_dt)
            g_ps = gpsum.tile([P, P], f32, tag="ga")
            for i in range(NT):
                nc.tensor.matmul(
                    g_ps[:],
                    lhsT=enc_sb[:, i, :],
                    rhs=enc_sb[:, i, :],
                    start=(i == 0),
                    stop=(i == NT - 1),
                )
            nc.vector.tensor_copy(g_sb[:], g_ps[:])
            g_parts = [g_sb]

        # ---- OutT = G @ DecT: wide moving chunks, stationary G ----
        # Pipeline: PE matmul(s) -> (DVE|ACT) PSUM->SBUF copy -> store.
        n_final = T // FINAL_N
        ncopy = FINAL_N // COPY_N
        for c in range(n_final):
            op = psum.tile([P, FINAL_N], f32, tag="op")
            rhs = dect_sb[:, c * FINAL_N : (c + 1) * FINAL_N]
            for h, gh in enumerate(g_parts):
                nc.tensor.matmul(
                    op[:],
                    lhsT=gh[:],
                    rhs=rhs,
                    start=(h == 0),
                    stop=(h == len(g_parts) - 1),
                )
            for k in range(ncopy):
                idx = c * ncopy + k
                lo = c * FINAL_N + k * COPY_N
                src = op[:, k * COPY_N : (k + 1) * COPY_N]
                if idx % 2 == 0:
                    nc.vector.tensor_copy(out_sb[:, lo : lo + COPY_N], src)
                else:
                    nc.scalar.copy(out_sb[:, lo : lo + COPY_N], src)
                deng = nc.sync if idx % 2 == 0 else nc.scalar
                deng.dma_start(
                    out=out_v[:, lo : lo + COPY_N],
                    in_=out_sb[:, lo : lo + COPY_N],
                )

    nc.compile()
    return nc


_NC = {}


def _get_nc(mm_dtype=None):
    mm_dtype = mm_dtype or MM_DTYPE
    if mm_dtype not in _NC:
        _NC[mm_dtype] = _build_nc(mm_dtype)
    return _NC[mm_dtype]


def _np_in_dtype(mm_dtype):
    if mm_dtype == "bf16":
        import ml_dtypes

        return ml_dtypes.bfloat16
    if mm_dtype == "fp16":
        return np.float16
    return np.float32


def _run(enc, dec, mm_dtype=None, **kwargs):
    mm_dtype = mm_dtype or MM_DTYPE
    nc = _get_nc(mm_dtype)
    np_dt = _np_in_dtype(mm_dtype)
    in_maps = []
    for b in range(B):
        in_maps.append(
            {
                "enc": np.ascontiguousarray(enc[b].astype(np_dt)),
                "dect": np.ascontiguousarray(dec[b].T.astype(np_dt)),
            }
        )
    res = run_bass_kernel_spmd(nc, in_maps, core_ids=list(range(B)), **kwargs)
    out = np.stack([res.results[b]["out"].T for b in range(B)], axis=0)
    return np.ascontiguousarray(out), res


def kernel(encoder_hidden_states, decoder_hidden_states):
    enc = np.ascontiguousarray(np.asarray(encoder_hidden_states, dtype=np.float32))
    dec = np.ascontiguousarray(np.asarray(decoder_hidden_states, dtype=np.float32))
    assert enc.shape == (B, T, D) and dec.shape == (B, T, D)
    out, _ = _run(enc, dec)
    return out


# revision 20
# speedup vs baseline: 1.1168x; 1.1168x over previous
"""Trainium2 Bass kernel for nn_LuongAttention.

Reference math (per batch b):
    S   = Dec @ Enc^T          # [T_dec, T_enc]
    Out = S @ Enc              # [T_dec, D]

By associativity:  Out = Dec @ (Enc^T @ Enc) = Dec @ G with G = Enc^T Enc
a [D, D] = [128, 128] Gram matrix.  This removes the [2048, 2048]
intermediate entirely (16x less FLOPs) and makes the kernel
memory-bound: ~3 MiB HBM I/O per core.

Sharding: data-parallel over batch B=8 -> one batch per NeuronCore.

Device-side layout trick: the host feeds Dec pre-transposed (DecT
[D, T]) and receives Out transposed (OutT [D, T]); the host transposes
the result back during the gather (pure layout permutation, no math).
With that:
  - G = sum_i EncTile_i^T @ EncTile_i  (accumulating PE matmuls, natural
    encoder layout - no transposes needed)
  - OutT = G @ DecT computed as matmul(lhsT=G, rhs=DecT chunk) with wide
    moving chunks (G is symmetric so lhsT=G gives G.T@X = G@X)
  - no PE transposes, no identity, minimal PSUM->SBUF copies

With SPLIT_G (2-byte dtypes), G is built as two half-Grams and the
final product accumulates OutT = Ga@DecT + Gb@DecT in PSUM, letting the
wide final matmuls start once the first half of the encoder has landed.
"""

import os
import sys
from contextlib import ExitStack

import numpy as np

for _p in (
    "/opt/trn_rl_repo",
    "/root/.axon_site",
    "/root/.axon_site/_ro/trn_rl_repo",
    "/root/.axon_site/_ro/pypackages",
):
    if os.path.isdir(_p) and _p not in sys.path:
        sys.path.append(_p)

import concourse.bacc as bacc
import concourse.mybir as mybir
import concourse.tile as tile
from concourse.bass_utils import run_bass_kernel_spmd

B, T, D, P = 8, 2048, 128, 128
NT = T // P  # 16 row tiles of 128

# tunables
MM_DTYPE = "fp16"  # "fp32" | "f32r" | "bf16" | "fp16"
ENC_CHUNKS = 8  # tiles per chunk = NT / ENC_CHUNKS
DEC_CHUNKS = 4
FINAL_N = 512  # moving-operand width of the final matmul
COPY_N = 512  # PSUM->SBUF copy granularity
STORE_N = 256  # store granularity
WARMUP_MMS = 0  # junk matmuls issued early to trigger the PE HAM clock ramp
SPLIT_G = False  # 2-step final accumulation: OutT = Ga@DecT + Gb@DecT


def _build_nc(mm_dtype=None):
    mm_dtype = mm_dtype or MM_DTYPE
    nc = bacc.Bacc("TRN2", target_bir_lowering=False, debug=False)
    f32 = mybir.dt.float32
    f32r = mybir.dt.float32r
    bf16 = mybir.dt.bfloat16
    fp16 = mybir.dt.float16

    # float32r is 4-byte fp32 storage with reduced-precision PE multiplies;
    # the BIR verifier requires every producer of an f32r matmul input to
    # carry the f32r dtype, so DRAM/SBUF tensors are declared f32r
    # end-to-end (numpy side stays np.float32).
    in_dt = {"bf16": bf16, "fp16": fp16, "f32r": f32r}.get(mm_dtype, f32)
    split_g = SPLIT_G and mm_dtype in ("fp16", "bf16")

    enc_h = nc.dram_tensor("enc", [T, D], in_dt, kind="ExternalInput")
    dect_h = nc.dram_tensor("dect", [D, T], in_dt, kind="ExternalInput")
    out_h = nc.dram_tensor("out", [D, T], f32, kind="ExternalOutput")

    # [p, n, d] view of encoder (p = row within tile, n = tile index)
    enc_v = enc_h.ap().rearrange("(n p) d -> p n d", p=P)
    dect_v = dect_h.ap()
    out_v = out_h.ap()

    with ExitStack() as ctx:
        tc = ctx.enter_context(tile.TileContext(nc))
        singles = ctx.enter_context(tc.tile_pool(name="singles", bufs=1))
        psum = ctx.enter_context(tc.tile_pool(name="psum", bufs=3, space="PSUM"))
        gpsum = ctx.enter_context(tc.tile_pool(name="gpsum", bufs=1, space="PSUM"))

        enc_sb = singles.tile([P, NT, D], in_dt)
        dect_sb = singles.tile([P, T], in_dt)
        out_sb = singles.tile([P, T], f32)

        if WARMUP_MMS:
            # Busy the PE while the first loads are in flight so the HAM
            # clock gate releases (1.2 -> 2.4 GHz) before the real matmuls.
            wsrc = singles.tile([P, 512], bf16)
            nc.gpsimd.memset(wsrc[:], 0.0)
            wps = gpsum.tile([P, 512], f32, tag="warm")
            for w in range(WARMUP_MMS):
                nc.tensor.matmul(
                    wps[:],
                    lhsT=wsrc[:, :P],
                    rhs=wsrc[:],
                    start=(w == 0),
                    stop=(w == WARMUP_MMS - 1),
                )
            wsink = singles.tile([P, 1], f32)
            nc.vector.tensor_copy(wsink[:], wps[:, :1])

        # Interleave chunked loads across both HWDGE rings (SP=sync,
        # ACT=scalar); encoder first on each ring since G consumes it first.
        cs = NT // ENC_CHUNKS
        for c in range(ENC_CHUNKS):
            eng = nc.sync if c % 2 == 0 else nc.scalar
            eng.dma_start(
                out=enc_sb[:, c * cs : (c + 1) * cs, :],
                in_=enc_v[:, c * cs : (c + 1) * cs, :],
            )
        cs = T // DEC_CHUNKS
        for c in range(DEC_CHUNKS):
            eng = nc.sync if c % 2 == 0 else nc.scalar
            eng.dma_start(
                out=dect_sb[:, c * cs : (c + 1) * cs],
                in_=dect_v[:, c * cs : (c + 1) * cs],
            )

        # ---- Gram matrix construction ----
        if split_g:
            H = NT // 2
            g_parts = []
            for h in range(2):
                ps = gpsum.tile([P, P], f32, tag=f"g{h}")
                for i in range(H):
                    nc.tensor.matmul(
                        ps[:],
                        lhsT=enc_sb[:, h * H + i, :],
                        rhs=enc_sb[:, h * H + i, :],
                        start=(i == 0),
                        stop=(i == H - 1),
                    )
                gh = singles.tile([P, P], in_dt, tag=f"gsb{h}")
                nc.vector.tensor_copy(gh[:], ps[:])
                g_parts.append(gh)
        elif mm_dtype == "f32r":
            # Pair tiles: rhs spans two adjacent tiles (N=256) so float32r
            # runs at 1 cycle/row instead of 4.  Useful halves:
            #   ps_a left  half accumulates even-tile Grams
            #   ps_b right half accumulates odd-tile Grams
            g_sb = singles.tile([P, P], in_dt)
            ps_a = gpsum.tile([P, 2 * P], f32, tag="ga")
            ps_b = gpsum.tile([P, 2 * P], f32, tag="gb")
            for pair in range(NT // 2):
                i, j = 2 * pair, 2 * pair + 1
                rhs = enc_sb[:, i : i + 2, :]
                nc.tensor.matmul(
                    ps_a[:],
                    lhsT=enc_sb[:, i, :],
                    rhs=rhs,
                    start=(pair == 0),
                    stop=(pair == NT // 2 - 1),
                )
                nc.tensor.matmul(
                    ps_b[:],
                    lhsT=enc_sb[:, j, :],
                    rhs=rhs,
                    start=(pair == 0),
                    stop=(pair == NT // 2 - 1),
                )
            # DVE may read only one PSUM operand per instruction.
            ga_sb = singles.tile([P, P], f32)
            nc.vector.tensor_copy(ga_sb[:], ps_a[:, :P])
            nc.vector.tensor_add(g_sb[:], ga_sb[:], ps_b[:, P:])
            g_parts = [g_sb]
        else:
            g_sb = singles.tile([P, P], in_dt)
            g_ps = gpsum.tile([P, P], f32, tag="ga")
            for i in range(NT):
                nc.tensor.matmul(
                    g_ps[:],
                    lhsT=enc_sb[:, i, :],
                    rhs=enc_sb[:, i, :],
                    start=(i == 0),
                    stop=(i == NT - 1),
                )
            nc.vector.tensor_copy(g_sb[:], g_ps[:])
            g_parts = [g_sb]

        # ---- OutT = G @ DecT: wide moving chunks, stationary G ----
        # Pipeline: PE matmul(s) -> (DVE|ACT) PSUM->SBUF copy -> store.
        n_final = T // FINAL_N
        ncopy = FINAL_N // COPY_N
        for c in range(n_final):
            op = psum.tile([P, FINAL_N], f32, tag="op")
            rhs = dect_sb[:, c * FINAL_N : (c + 1) * FINAL_N]
            for h, gh in enumerate(g_parts):
                nc.tensor.matmul(
                    op[:],
                    lhsT=gh[:],
                    rhs=rhs,
                    start=(h == 0),
                    stop=(h == len(g_parts) - 1),
                )
            for k in range(ncopy):
                idx = c * ncopy + k
                lo = c * FINAL_N + k * COPY_N
                src = op[:, k * COPY_N : (k + 1) * COPY_N]
                if idx % 2 == 0:
                    nc.vector.tensor_copy(out_sb[:, lo : lo + COPY_N], src)
                else:
                    nc.scalar.copy(out_sb[:, lo : lo + COPY_N], src)
                deng = nc.sync if idx % 2 == 0 else nc.scalar
                deng.dma_start(
                    out=out_v[:, lo : lo + COPY_N],
                    in_=out_sb[:, lo : lo + COPY_N],
                )

    nc.compile()
    return nc


_NC = {}


def _get_nc(mm_dtype=None):
    mm_dtype = mm_dtype or MM_DTYPE
    if mm_dtype not in _NC:
        _NC[mm_dtype] = _build_nc(mm_dtype)
    return _NC[mm_dtype]


def _np_in_dtype(mm_dtype):
    if mm_dtype == "bf16":
        import ml_dtypes

        return ml_dtypes.bfloat16
    if mm_dtype == "fp16":
        return np.float16
    return np.float32


def _run(enc, dec, mm_dtype=None, **kwargs):
    mm_dtype = mm_dtype or MM_DTYPE
    nc = _get_nc(mm_dtype)
    np_dt = _np_in_dtype(mm_dtype)
    in_maps = []
    for b in range(B):
        in_maps.append(
            {
                "enc": np.ascontiguousarray(enc[b].astype(np_dt)),
                "dect": np.ascontiguousarray(dec[b].T.astype(np_dt)),
            }
        )
    res = run_bass_kernel_spmd(nc, in_maps, core_ids=list(range(B)), **kwargs)
    out = np.stack([res.results[b]["out"].T for b in range(B)], axis=0)
    return np.ascontiguousarray(out), res


def kernel(encoder_hidden_states, decoder_hidden_states):
    enc = np.ascontiguousarray(np.asarray(encoder_hidden_states, dtype=np.float32))
    dec = np.ascontiguousarray(np.asarray(decoder_hidden_states, dtype=np.float32))
    assert enc.shape == (B, T, D) and dec.shape == (B, T, D)
    out, _ = _run(enc, dec)
    return out
